# revision 8
# baseline (speedup 1.0000x reference)
"""Embedding-lookup MF model kernel for Trainium2 (8 NeuronCores).

reference math (B = 16384, D = 64):
    u   = user_table[x[:, 0]]          # [B, D]
    v   = item_table[x[:, 1]]          # [B, D]
    out = sigmoid(sum(u * v, -1))      # [B]

Strategy: data-parallel across the batch (2048 pairs per core). Tables live
in HBM as f32 granules; rows are fetched with the SWDGE dma_gather ucode
instruction and the in-granule row is selected on the DVE.

Why this shape: SWDGE descriptor generation on the Q7 (Pool/GpSimd engine)
runs at ~8.5 ns per descriptor plus ~1 us fixed per instruction, and
descriptors == indices. The baseline's 32 indirect-DMA instructions paid the
fixed cost 32x (45 us chain). dma_gather packs 1024 indices per instruction
(the HW cap; 2048 crashes the exec unit), cutting the chain to 4
instructions (~36 us). The ~8.5 ns/desc Q7 rate is the hard floor for any
scattered gather on this hardware — measured identical for indirect-DMA and
dma_gather ucode paths.

dma_gather indices are int16 (signed, 15 usable bits -> max 32767), but the
tables have 100K rows. Solution: 4-row granules. Tables are uploaded as
[25000, 256] f32 (granule g = rows 4g..4g+3 flattened); the gather index is
id>>2 (< 25000, fits int16) and fetches the whole 1-KB granule. The wanted
row (s = id & 3) is selected on the DVE with a host-built one-hot f32 mask
(exact math — mask multiply and adds of zero introduce no rounding):
    wu   = granule * mask
    usel = tree-add over the 4 rows
    out  = sigmoid(reduce_add(usel * vsel))
f32 throughout keeps max abs err ~1e-6 (bf16 tables measured 1.8e-2 abs /
7e-2 rel — too close to the 2e-2 gate).

Raw Block style (explicit semaphores, no Tile scheduler): the Tile version
measured two pathologies — dma_gather instructions stalling ~4 us each on
SWDGE descriptor-ring space (default 16 KB ring == exactly one 1024-idx
instruction; fixed via dynamic_dma_scratch_size=65536), and the DVE stream
scheduled out of data-arrival order (24 us idle). Here every engine's
program order is the intended execution order.

Layouts (per core, P=128, NBLK=16, 2 chunks of 1024 pairs):
    pair j -> partition j%128, block j//128 (gather output and result slot)
    idx tile [128, 256] int16: cols [0,128) u-granule ids, [128,256) v;
      index i of chunk c at (i%16, c*64 + i//16), replicated over the 8
      16-partition stripes (dma_gather wrap-16 layout, verified on HW)
    masks [128, 16, 4, 64] f32, expanded over D host-side so every DVE
      operand has a packed last dim
"""

import os

# A previously crashed process can leave the NeuronCores wedged
# (NRT_EXEC_UNIT_UNRECOVERABLE on the next run); requesting a core reset at
# runtime init is harmless otherwise and self-heals that state.
os.environ.setdefault("NEURON_RT_RESET_CORES", "1")

import numpy as np

import concourse.bass as bass
import concourse.mybir as mybir
from concourse import bacc
from concourse.bass_utils import run_bass_kernel_spmd

N_CORES = 8
P = 128
D = 64
B = 16384
BPC = B // N_CORES  # 2048 pairs per core
NBLK = BPC // P  # 16 result blocks of 128 pairs
CHUNK = 1024  # dma_gather HW cap per instruction
NCH = BPC // CHUNK  # 2 chunks
CBLK = CHUNK // P  # 8 blocks per chunk
GR = 4  # rows per granule
TROWS = 100_000  # table rows actually addressable (ids < 100000)
GROWS = TROWS // GR  # 25000 granules
GCOLS = GR * D  # 256 f32 elements per granule

_programs: dict = {}


def _build():
    """Build the single-core program (run SPMD on 8 cores)."""
    nc = bacc.Bacc(
        "TRN2",
        target_bir_lowering=False,
        debug=False,
        detect_race_conditions=False,
        # SWDGE descriptor-ring carveout: default 16384 B holds exactly 1024
        # descriptors == one gather instruction, serializing instruction N+1
        # behind instruction N's DMA transfers (~4 us each measured). 64 KB
        # holds all 4096 in-flight descriptors.
        dynamic_dma_scratch_size=65536,
    )
    f32, bf16, i16 = mybir.dt.float32, mybir.dt.bfloat16, mybir.dt.int16
    uvidx = nc.dram_tensor("uvidx", [P, 2 * NBLK * 8], i16, kind="ExternalInput")
    um = nc.dram_tensor("um", [P, NBLK, GR, D], f32, kind="ExternalInput")
    vm = nc.dram_tensor("vm", [P, NBLK, GR, D], f32, kind="ExternalInput")
    ut = nc.dram_tensor("ut", [GROWS, GCOLS], f32, kind="ExternalInput")
    it = nc.dram_tensor("it", [GROWS, GCOLS], f32, kind="ExternalInput")
    out = nc.dram_tensor("out", [P, NBLK], f32, kind="ExternalOutput")

    from contextlib import ExitStack

    with (
        nc.Block() as block,
        nc.sbuf_tensor("t_idx", [P, 2 * NBLK * 8], i16) as t_idx,
        nc.sbuf_tensor("t_um", [P, NBLK, GR, D], f32) as t_um,
        nc.sbuf_tensor("t_vm", [P, NBLK, GR, D], f32) as t_vm,
        nc.sbuf_tensor("tgu", [P, NBLK, GCOLS], f32) as tgu,
        nc.sbuf_tensor("tgv", [P, NBLK, GCOLS], f32) as tgv,
        nc.sbuf_tensor("wsel", [P, NCH, 2, CBLK, GR * D], f32) as wsel,
        nc.sbuf_tensor("sel", [P, NCH, 2, CBLK, 3 * D], f32) as sel,
        nc.sbuf_tensor("t_res", [P, NBLK], f32) as t_res,
        nc.sbuf_tensor("t_bias", [P, 1], f32) as t_bias,
        nc.semaphore("s_idx") as s_idx,  # idx input DMA (+16)
        nc.semaphore("s_m") as s_m,  # mask input DMAs (+16 each)
        nc.semaphore("s_g") as s_g,  # gather DMA completions (+16 each)
        nc.semaphore("s_v") as s_v,  # DVE chain ticks
        nc.semaphore("s_a") as s_a,  # sigmoid done per chunk
        nc.semaphore("s_o") as s_o,  # output stores
    ):
        ES = ExitStack()  # noqa: F841  (kept for structural parity)

        @block.sync
        def _(sync: bass.BassEngine):
            # idx first — it gates the whole gather chain; masks can land
            # any time before the DVE selects.
            sync.dma_start(t_idx[:], uvidx[:]).then_inc(s_idx, 16)
            sync.dma_start(t_um[:], um[:]).then_inc(s_m, 16)
            sync.dma_start(t_vm[:], vm[:]).then_inc(s_m, 16)
            # output stores, one per chunk
            for c in range(NCH):
                sync.wait_ge(s_a, c + 1)
                sync.dma_start(
                    out[:, c * CBLK : (c + 1) * CBLK],
                    t_res[:, c * CBLK : (c + 1) * CBLK],
                ).then_inc(s_o, 16)
            sync.wait_ge(s_o, 16 * NCH)

        @block.gpsimd
        def _(gpsimd: bass.BassGpSimd):
            gpsimd.wait_ge(s_idx, 16)  # idx tile landed
            for c in range(NCH):
                i0, i1 = c * 64, (c + 1) * 64
                gpsimd.dma_gather(
                    tgu[:, c * CBLK : (c + 1) * CBLK, :], ut[:],
                    t_idx[:, i0:i1], CHUNK, CHUNK, GCOLS,
                ).then_inc(s_g, 16)
                gpsimd.dma_gather(
                    tgv[:, c * CBLK : (c + 1) * CBLK, :], it[:],
                    t_idx[:, 128 + i0 : 128 + i1], CHUNK, CHUNK, GCOLS,
                ).then_inc(s_g, 16)

        @block.vector
        def _(vector: bass.BassVectorEngine):
            vector.memset(t_bias[:], 0.0)
            vector.wait_ge(s_m, 32)  # masks landed
            for c in range(NCH):
                b0, b1 = c * CBLK, (c + 1) * CBLK
                for side in range(2):  # 0 = u, 1 = v
                    tg = (tgu, tgv)[side]
                    tm = (t_um, t_vm)[side]
                    # gather DMAs complete in issue order: u0,v0,u1,v1
                    vector.wait_ge(s_g, 16 * (2 * c + side + 1))
                    w = wsel[:, c, side]  # [P, CBLK, GR*D]
                    vector.tensor_mul(
                        out=w[:].rearrange("p n (s d) -> p n s d", d=D),
                        in0=tg[:, b0:b1, :].rearrange("p n (s d) -> p n s d", d=D),
                        in1=tm[:, b0:b1],
                    ).then_inc(s_v, 1)
                    a2 = sel[:, c, side, :, 0 : 2 * D]  # [P, CBLK, 2D]
                    vector.tensor_add(
                        out=a2, in0=w[:, :, 0 : 2 * D], in1=w[:, :, 2 * D : 4 * D]
                    ).then_inc(s_v, 1)
                    a1 = sel[:, c, side, :, 2 * D : 3 * D]  # [P, CBLK, D]
                    vector.tensor_add(
                        out=a1, in0=a2[:, :, 0:D], in1=a2[:, :, D : 2 * D]
                    ).then_inc(s_v, 1)
                # dot product + reduce, f32
                prod = wsel[:, c, 0, :, 0:D]  # reuse wu scratch
                vector.tensor_mul(
                    out=prod,
                    in0=sel[:, c, 0, :, 2 * D : 3 * D],
                    in1=sel[:, c, 1, :, 2 * D : 3 * D],
                ).then_inc(s_v, 1)
                vector.reduce_sum(
                    out=t_res[:, b0:b1],
                    in_=prod.rearrange("p n d -> p n d"),
                    axis=mybir.AxisListType.X,
                ).then_inc(s_v, 1)

        @block.scalar
        def _(scalar: bass.BassScalarEngine):
            for c in range(NCH):
                scalar.wait_ge(s_v, 8 * (c + 1))  # chunk's reduce done
                rs = t_res[:, c * CBLK : (c + 1) * CBLK]
                scalar.activation(
                    out=rs,
                    in_=rs,
                    func=mybir.ActivationFunctionType.Sigmoid,
                    bias=t_bias[:],
                ).then_inc(s_a, 1)

    nc.compile()
    return nc


def _get_program():
    if "p" not in _programs:
        _programs["p"] = _build()
    return _programs["p"]


def _wrap16(q: np.ndarray) -> np.ndarray:
    """[BPC] granule ids -> [128, 128] int16 dma_gather idx tile
    (index i at partition i%16, col i//16; replicated over 8 stripes)."""
    w = q.reshape(BPC // 16, 16).T.astype(np.int16)  # [16, 128]
    return np.tile(w, (8, 1))


def _onehot_mask(s: np.ndarray) -> np.ndarray:
    """[BPC] row-in-granule -> [128, NBLK, 4, 64] f32 one-hot, expanded
    over D host-side so the DVE multiply sees a packed operand."""
    m = (s[:, None] == np.arange(GR)).astype(np.float32)  # [BPC, 4]
    m = m.reshape(NBLK, P, GR).transpose(1, 0, 2)  # [128, 16, 4]
    return np.ascontiguousarray(np.broadcast_to(m[..., None], (P, NBLK, GR, D)))


def _prep_core_inputs(xs: np.ndarray, ut_g: np.ndarray, it_g: np.ndarray) -> dict:
    uid, vid = xs[:, 0], xs[:, 1]
    uvidx = np.concatenate([_wrap16(uid >> 2), _wrap16(vid >> 2)], axis=1)
    return {
        "uvidx": np.ascontiguousarray(uvidx),
        "um": _onehot_mask(uid & (GR - 1)),
        "vm": _onehot_mask(vid & (GR - 1)),
        "ut": ut_g,
        "it": it_g,
    }


def _prep_tables(user_table: np.ndarray, item_table: np.ndarray):
    ut_g = np.ascontiguousarray(
        user_table[:TROWS].astype(np.float32).reshape(GROWS, GCOLS)
    )
    it_g = np.ascontiguousarray(
        item_table[:TROWS].astype(np.float32).reshape(GROWS, GCOLS)
    )
    return ut_g, it_g


def _run(x, user_table, item_table, **run_kwargs):
    x = np.asarray(x)
    ut = np.asarray(user_table, dtype=np.float32)
    it = np.asarray(item_table, dtype=np.float32)
    assert x.shape == (B, 2), x.shape
    xi = x.astype(np.int64)
    assert xi.min() >= 0 and xi.max() < TROWS, (xi.min(), xi.max())
    ut_g, it_g = _prep_tables(ut, it)
    nc = _get_program()
    in_maps = [
        _prep_core_inputs(xi[k * BPC : (k + 1) * BPC], ut_g, it_g)
        for k in range(N_CORES)
    ]
    res = run_bass_kernel_spmd(nc, in_maps, list(range(N_CORES)), **run_kwargs)
    out = np.empty(B, np.float32)
    for k in range(N_CORES):
        out[k * BPC : (k + 1) * BPC] = res.results[k]["out"].T.ravel()
    return out, res


def kernel(x, user_table, item_table):
    out, _ = _run(x, user_table, item_table)
    return out


# revision 9
# speedup vs baseline: 1.4351x; 1.4351x over previous
"""Embedding-lookup MF model kernel for Trainium2 (8 NeuronCores).

reference math (B = 16384, D = 64):
    u   = user_table[x[:, 0]]          # [B, D]
    v   = item_table[x[:, 1]]          # [B, D]
    out = sigmoid(sum(u * v, -1))      # [B]

Strategy: data-parallel across the batch. Each of the 8 cores handles 2048
batch rows. The two tables are concatenated host-side into one [U+I, D]
table (user ids produced by the reference's randint fill are < 100000, so
only that prefix of the 1M-row user table is ever referenced; we upload a
prefix sized to the actual max id).

The TRN2 indirect-DMA primitive consumes exactly ONE index per destination
partition and fills that partition's dest extent contiguously from
table[idx[p]] (verified on HW). So each gather instruction moves 128 rows:
dest [128, 64] slice, offsets [128, 1]. 2048 u-rows + 2048 v-rows per core
= 32 gather instructions, pipelined with the DVE mul + segmented-reduce and
ACT sigmoid per chunk.

Layout per core (P=128 partitions, NBLK=16 blocks):
    batch row  b = n*128 + p   lives at  partition p, block n
    idx  SBUF tile [128, 32] int32: col n       = u-id of block n
                                    col 16 + n  = (u_rows + v-id) of block n
    gather tile tg [128, 2048] f32: u rows at cols [0,1024), v at [1024,2048)
"""

import os

# A previously crashed process can leave the NeuronCores wedged
# (NRT_EXEC_UNIT_UNRECOVERABLE on the next run); requesting a core reset at
# runtime init is harmless otherwise and self-heals that state.
os.environ.setdefault("NEURON_RT_RESET_CORES", "1")

import numpy as np

import concourse.bass as bass
import concourse.mybir as mybir
import concourse.tile as tile
from concourse import bacc
from concourse.bass_utils import run_bass_kernel_spmd

N_CORES = 8
P = 128
D = 64
B = 16384
BPC = B // N_CORES  # 2048 batch rows per core
NBLK = BPC // P  # 16 column blocks of 128 batch rows
# Tapered chunking: desc-gen for all 32 gathers is serial on the Q7, so only
# the LAST chunk's DMA-receipt + mul/reduce/sigmoid/store chain is exposed at
# the tail. Keep the last chunk minimal.
CHUNK_BLOCKS = [5, 5, 5, 1]

_programs: dict = {}


def _build(cat_rows: int):
    """Build the single-core program (run SPMD on 8 cores)."""
    nc = bacc.Bacc(
        "TRN2",
        target_bir_lowering=False,
        debug=False,
        detect_race_conditions=False,
        # SWDGE descriptor-ring carveout: the default 16384 B holds 1024
        # descriptors, so with 32x128-desc indirect gathers in flight the
        # ring fills after 8 instructions and each later instruction stalls
        # ~300 ns on reclaim. 64 KB holds all 4096 descriptors.
        dynamic_dma_scratch_size=65536,
    )
    idx = nc.dram_tensor("idx", [P, 2 * NBLK], mybir.dt.int32, kind="ExternalInput")
    tbl = nc.dram_tensor("tbl", [cat_rows, D], mybir.dt.float32, kind="ExternalInput")
    out = nc.dram_tensor("out", [P, NBLK], mybir.dt.float32, kind="ExternalOutput")

    with tile.TileContext(nc) as tc:
        with (
            tc.tile_pool(name="io", bufs=1) as io_pool,
            tc.tile_pool(name="prod", bufs=2) as prod_pool,
        ):
            t_idx = io_pool.tile([P, 2 * NBLK], mybir.dt.int32)
            nc.sync.dma_start(out=t_idx[:], in_=idx[:])
            tg = io_pool.tile([P, 2 * NBLK * D], mybir.dt.float32)
            t_res = io_pool.tile([P, NBLK], mybir.dt.float32)
            # zero bias tile for the sigmoid activation: avoids the const-AP
            # DMA the framework would otherwise emit ahead of the idx load
            t_bias = io_pool.tile([P, 1], mybir.dt.float32)
            nc.vector.memset(t_bias[:], 0.0)
            b0 = 0
            for nb in CHUNK_BLOCKS:
                b1 = b0 + nb
                # gather this chunk's u blocks and v blocks, one row per
                # partition per instruction
                for j in list(range(b0, b1)) + list(range(NBLK + b0, NBLK + b1)):
                    nc.gpsimd.indirect_dma_start(
                        out=tg[:, j * D : (j + 1) * D],
                        out_offset=None,
                        in_=tbl[:],
                        in_offset=bass.IndirectOffsetOnAxis(
                            ap=t_idx[:, j : j + 1], axis=0
                        ),
                    )
                w = prod_pool.tile([P, nb * D], mybir.dt.float32, tag="w")
                nc.vector.tensor_mul(
                    out=w[:],
                    in0=tg[:, b0 * D : b1 * D],
                    in1=tg[:, (NBLK + b0) * D : (NBLK + b1) * D],
                )
                rs = t_res[:, b0:b1]
                nc.vector.reduce_sum(
                    out=rs,
                    in_=w[:].rearrange("p (n d) -> p n d", d=D),
                    axis=mybir.AxisListType.X,
                )
                nc.scalar.activation(
                    out=rs,
                    in_=rs,
                    func=mybir.ActivationFunctionType.Sigmoid,
                    bias=t_bias[:],
                )
                # store each chunk as soon as its sigmoid lands; only the last
                # (1-block) store sits on the critical tail
                nc.sync.dma_start(out=out[:, b0:b1], in_=t_res[:, b0:b1])
                b0 = b1
    nc.compile()
    return nc


def _get_program(cat_rows: int):
    if cat_rows not in _programs:
        _programs[cat_rows] = _build(cat_rows)
    return _programs[cat_rows]


def _prep_idx(xs: np.ndarray, u_rows: int) -> np.ndarray:
    """[BPC, 2] int32 -> [128, 32] idx tile (u cols then offset v cols)."""
    iu = xs[:, 0].reshape(NBLK, P).T  # [P, NBLK]
    iv = xs[:, 1].reshape(NBLK, P).T + u_rows
    return np.ascontiguousarray(np.concatenate([iu, iv], axis=1), dtype=np.int32)


def _run(x, user_table, item_table, **run_kwargs):
    x = np.asarray(x)
    ut = np.asarray(user_table, dtype=np.float32)
    it = np.asarray(item_table, dtype=np.float32)
    assert x.shape == (B, 2), x.shape
    xi = x.astype(np.int32)
    # user ids from the reference's randint fill are < 100000; upload only
    # the prefix of the user table that can actually be referenced.
    u_rows = min(ut.shape[0], max(100_000, int(xi[:, 0].max()) + 1))
    cat = np.ascontiguousarray(np.concatenate([ut[:u_rows], it], axis=0))
    nc = _get_program(cat.shape[0])
    in_maps = []
    for k in range(N_CORES):
        xs = xi[k * BPC : (k + 1) * BPC]
        in_maps.append({"idx": _prep_idx(xs, u_rows), "tbl": cat})
    res = run_bass_kernel_spmd(nc, in_maps, list(range(N_CORES)), **run_kwargs)
    out = np.empty(B, np.float32)
    for k in range(N_CORES):
        out[k * BPC : (k + 1) * BPC] = res.results[k]["out"].T.ravel()
    return out, res


def kernel(x, user_table, item_table):
    out, _ = _run(x, user_table, item_table)
    return out



# revision 10
# speedup vs baseline: 1.4392x; 1.0028x over previous
"""Embedding-lookup MF model kernel for Trainium2 (8 NeuronCores).

reference math (B = 16384, D = 64):
    u   = user_table[x[:, 0]]          # [B, D]
    v   = item_table[x[:, 1]]          # [B, D]
    out = sigmoid(sum(u * v, -1))      # [B]

Strategy: data-parallel across the batch. Each of the 8 cores handles 2048
batch rows. The two tables are concatenated host-side into one [U+I, D]
table (user ids produced by the reference's randint fill are < 100000, so
only that prefix of the 1M-row user table is ever referenced; we upload a
prefix sized to the actual max id).

The TRN2 indirect-DMA primitive consumes exactly ONE index per destination
partition and fills that partition's dest extent contiguously from
table[idx[p]] (verified on HW). So each gather instruction moves 128 rows:
dest [128, 64] slice, offsets [128, 1]. 2048 u-rows + 2048 v-rows per core
= 32 gather instructions, pipelined with the DVE mul + segmented-reduce and
ACT sigmoid per chunk.

Layout per core (P=128 partitions, NBLK=16 blocks):
    batch row  b = n*128 + p   lives at  partition p, block n
    idx  SBUF tile [128, 32] int32: col n       = u-id of block n
                                    col 16 + n  = (u_rows + v-id) of block n
    gather tile tg [128, 2048] f32: u rows at cols [0,1024), v at [1024,2048)
"""

import os

# A previously crashed process can leave the NeuronCores wedged
# (NRT_EXEC_UNIT_UNRECOVERABLE on the next run); requesting a core reset at
# runtime init is harmless otherwise and self-heals that state.
os.environ.setdefault("NEURON_RT_RESET_CORES", "1")

import numpy as np

import concourse.bass as bass
import concourse.mybir as mybir
import concourse.tile as tile
from concourse import bacc
from concourse.bass_utils import run_bass_kernel_spmd

N_CORES = 8
P = 128
D = 64
B = 16384
BPC = B // N_CORES  # 2048 batch rows per core
NBLK = BPC // P  # 16 column blocks of 128 batch rows
# Tapered chunking: desc-gen for all 32 gathers is serial on the Q7, so only
# the LAST chunk's DMA-receipt + mul/reduce/sigmoid/store chain is exposed at
# the tail. Keep the last chunk minimal.
CHUNK_BLOCKS = [5, 5, 5, 1]

_programs: dict = {}


def _build(cat_rows: int):
    """Build the single-core program (run SPMD on 8 cores)."""
    nc = bacc.Bacc(
        "TRN2",
        target_bir_lowering=False,
        debug=False,
        detect_race_conditions=False,
        # SWDGE descriptor-ring carveout: the default 16384 B holds 1024
        # descriptors, so with 32x128-desc indirect gathers in flight the
        # ring fills after 8 instructions and each later instruction stalls
        # ~300 ns on reclaim. 64 KB holds all 4096 descriptors.
        dynamic_dma_scratch_size=65536,
    )
    idx = nc.dram_tensor("idx", [P, 2 * NBLK], mybir.dt.int32, kind="ExternalInput")
    tbl = nc.dram_tensor("tbl", [cat_rows, D], mybir.dt.float32, kind="ExternalInput")
    out = nc.dram_tensor("out", [P, NBLK], mybir.dt.float32, kind="ExternalOutput")

    with tile.TileContext(nc) as tc:
        with (
            tc.tile_pool(name="io", bufs=1) as io_pool,
            tc.tile_pool(name="prod", bufs=2) as prod_pool,
        ):
            t_idx = io_pool.tile([P, 2 * NBLK], mybir.dt.int32)
            # Load the idx tile from the gpsimd queue: its preamble drains
            # ~0.7 us before sync's, and the first gather's wait then rides
            # the same engine's DMA sem — the whole chain starts ~1 us
            # earlier than with the sync-issued load.
            nc.gpsimd.dma_start(out=t_idx[:], in_=idx[:])
            tg = io_pool.tile([P, 2 * NBLK * D], mybir.dt.float32)
            t_res = io_pool.tile([P, NBLK], mybir.dt.float32)
            # zero bias tile for the sigmoid activation: avoids the const-AP
            # DMA the framework would otherwise emit ahead of the idx load
            t_bias = io_pool.tile([P, 1], mybir.dt.float32)
            nc.vector.memset(t_bias[:], 0.0)
            b0 = 0
            for nb in CHUNK_BLOCKS:
                b1 = b0 + nb
                # gather this chunk's u blocks and v blocks, one row per
                # partition per instruction
                for j in list(range(b0, b1)) + list(range(NBLK + b0, NBLK + b1)):
                    nc.gpsimd.indirect_dma_start(
                        out=tg[:, j * D : (j + 1) * D],
                        out_offset=None,
                        in_=tbl[:],
                        in_offset=bass.IndirectOffsetOnAxis(
                            ap=t_idx[:, j : j + 1], axis=0
                        ),
                    )
                w = prod_pool.tile([P, nb * D], mybir.dt.float32, tag="w")
                nc.vector.tensor_mul(
                    out=w[:],
                    in0=tg[:, b0 * D : b1 * D],
                    in1=tg[:, (NBLK + b0) * D : (NBLK + b1) * D],
                )
                rs = t_res[:, b0:b1]
                nc.vector.reduce_sum(
                    out=rs,
                    in_=w[:].rearrange("p (n d) -> p n d", d=D),
                    axis=mybir.AxisListType.X,
                )
                nc.scalar.activation(
                    out=rs,
                    in_=rs,
                    func=mybir.ActivationFunctionType.Sigmoid,
                    bias=t_bias[:],
                )
                # store each chunk as soon as its sigmoid lands; only the last
                # (1-block) store sits on the critical tail
                nc.sync.dma_start(out=out[:, b0:b1], in_=t_res[:, b0:b1])
                b0 = b1
    nc.compile()
    return nc


def _get_program(cat_rows: int):
    if cat_rows not in _programs:
        _programs[cat_rows] = _build(cat_rows)
    return _programs[cat_rows]


def _prep_idx(xs: np.ndarray, u_rows: int) -> np.ndarray:
    """[BPC, 2] int32 -> [128, 32] idx tile (u cols then offset v cols)."""
    iu = xs[:, 0].reshape(NBLK, P).T  # [P, NBLK]
    iv = xs[:, 1].reshape(NBLK, P).T + u_rows
    return np.ascontiguousarray(np.concatenate([iu, iv], axis=1), dtype=np.int32)


def _run(x, user_table, item_table, **run_kwargs):
    x = np.asarray(x)
    ut = np.asarray(user_table, dtype=np.float32)
    it = np.asarray(item_table, dtype=np.float32)
    assert x.shape == (B, 2), x.shape
    xi = x.astype(np.int32)
    # user ids from the reference's randint fill are < 100000; upload only
    # the prefix of the user table that can actually be referenced.
    u_rows = min(ut.shape[0], max(100_000, int(xi[:, 0].max()) + 1))
    cat = np.ascontiguousarray(np.concatenate([ut[:u_rows], it], axis=0))
    nc = _get_program(cat.shape[0])
    in_maps = []
    for k in range(N_CORES):
        xs = xi[k * BPC : (k + 1) * BPC]
        in_maps.append({"idx": _prep_idx(xs, u_rows), "tbl": cat})
    res = run_bass_kernel_spmd(nc, in_maps, list(range(N_CORES)), **run_kwargs)
    out = np.empty(B, np.float32)
    for k in range(N_CORES):
        out[k * BPC : (k + 1) * BPC] = res.results[k]["out"].T.ravel()
    return out, res


def kernel(x, user_table, item_table):
    out, _ = _run(x, user_table, item_table)
    return out

